# revision 1
# baseline (speedup 1.0000x reference)
"""DetectionLoss Bass/Tile kernel for TRN2 (one core = one image; SPMD x8).

Algorithm per core (image b):
  Phase A (j-loop over G=32 gts, split DVE/GPSIMD by j):
    per-anchor running max IoU (maxv) + 1-based argmax (bestp).
  Phase B (two column halves):
    pos = maxv >= 0.5; bestp_m = pos * bestp
    one-hot(bestp_m) -> PE transpose -> PE matmul vs block-diag gt table
      => gathered xg, yg, ln(wg), ln(hg), one-hot(label)*pos   (all pre-masked)
    smooth-L1 on encoded reg targets; focal via Exp/Ln pipeline.
  Output: [npos, sl1_sum, Nsum, corr] partial sums; host finishes.

Layout: anchor a <-> (partition p = a // COLS, col f = a % COLS).
Inputs (per core, planar, host-packed):
  anch [4, A] f32, clsp [8, A] f32, regp [4, A] f32, gtaux [1, 160] f32
  (gtaux = gx1[32] gy1[32] gx2[32] gy2[32] labelf[32])
Output: out [1, 4] f32.
"""
import dataclasses
import numpy as np

import concourse.bass as bass
import concourse.mybir as mybir
from concourse import tile

AL = mybir.AluOpType
AF = mybir.ActivationFunctionType
f32 = mybir.dt.float32

P = 128
G = 32
C = 8
EPS = 1e-7
BETA = 1.0 / 9.0
POS_IOU = 0.5


def patch_tile_drain(maxw: int = 1):
    """Split the TileContext exit drain's sem waits across NOPs (walrus
    setupSyncWait rejects >1 wait on a CTRL instruction in this build)."""
    import concourse.tile as tile_mod
    from concourse.vector_clock import ScopedClock

    def _drain_and_barrier(self, tick_clock, wait_clock):
        drain_inst = self.nc.sync.drain()
        wait_clock.add_sem_waits(
            drain_inst.ins, ScopedClock({None: tick_clock.global_clock})
        )
        si = drain_inst.ins.sync_info
        waits = list(si.on_wait)
        if len(waits) > maxw:
            si.on_wait = waits[:maxw]
            rest = waits[maxw:]
            for i in range(0, len(rest), maxw):
                nop = self.nc.sync.nop(nofuse=True, hint="drain_split")
                nop.ins.sync_info = mybir.SyncInfo(
                    on_wait=rest[i:i + maxw], on_update=[]
                )
        self.nc.all_engine_barrier()
        assert self.sems is not None
        popped = self.nc._tile_sem_poison_stack.pop()
        assert popped is self._sem_poison
        self.nc.clear_and_free_semaphores(list(self.sems.allocated().values()))
        self.nc.all_engine_barrier()

    tile_mod.TileContext._drain_and_barrier = _drain_and_barrier


def split_sync_waits(nc, maxw: int = 1):
    """Walrus rejects >2 sem waits on one instruction (and >1 on CTRL-type).
    Hoist excess waits onto same-engine NOPs inserted immediately before."""
    ctr = [0]

    def mknop(engine, waits):
        ctr[0] += 1
        nop = mybir.InstNoOp(name=f"I-wsplit-{ctr[0]}", ins=[], outs=[])
        nop.engine = engine
        nop.sync_info = mybir.SyncInfo(on_wait=waits, on_update=[])
        return nop

    for blk in nc.bb_map.values():
        bb = blk.bb
        il = bb.instructions
        i = 0
        while i < len(il):
            inst = il[i]
            si = inst.sync_info
            mw = 1 if isinstance(inst, mybir.InstTensorScalarPtr) else maxw
            if si is not None and len(si.on_wait) > mw:
                waits = list(si.on_wait)
                si.on_wait = waits[:mw]
                rest = waits[mw:]
                for k in range(0, len(rest), 1):
                    il.insert(i, mknop(inst.engine, rest[k:k + 1]))
                    i += 1
            i += 1


def _expand32(ap):
    """[P, n] AP -> [P, n, 32] with step-0 inner dim (broadcast)."""
    return dataclasses.replace(ap, ap=ap.ap + [[0, G]])


def build(A: int, cg: int = 640, logchain: bool = False, repeat: int = 1):
    """Emit the per-core program. A must be divisible by 128.
    cg: GPSIMD handles anchor-columns [0, cg); DVE handles [cg, COLS)."""
    assert A % P == 0
    COLS = A // P
    NSQ = (COLS + 15) // 16          # superquads (16 cols each)
    if NSQ % 2:
        NSQ += 1                      # want two equal halves
    W = NSQ * 16
    HS = NSQ // 2                     # superquads per half
    WH = W // 2                       # padded cols per half
    cg = min(cg, COLS)
    LN_THIRD = float(np.log(np.float32(1.0) / np.float32(3.0)))
    THIRD = float(np.float32(1.0) / np.float32(3.0))

    nc = bass.Bass()
    anch = nc.declare_dram_parameter("anch", [4, A], f32, isOutput=False)
    clsp = nc.declare_dram_parameter("clsp", [C, A], f32, isOutput=False)
    regp = nc.declare_dram_parameter("regp", [4, A], f32, isOutput=False)
    gtaux = nc.declare_dram_parameter("gtaux", [1, 5 * G], f32, isOutput=False)
    out = nc.declare_dram_parameter("out", [1, 4], f32, isOutput=True)

    def plane(t, c):
        # [n, A] dram plane c -> [P, COLS] AP
        return t[c].rearrange("(p w) -> p w", p=P)

    with tile.TileContext(nc) as tc:
        from contextlib import ExitStack
        for _rep in range(repeat):
          with ExitStack() as ctx:
              const = ctx.enter_context(tc.tile_pool(name="const", bufs=1))
              persist = ctx.enter_context(tc.tile_pool(name="persist", bufs=1))

              # ---------- constants ----------
              iotarep = const.tile([P, 512], f32, name="iotarep")
              nc.gpsimd.iota(iotarep[:], pattern=[[0, 16], [1, G]], base=1,
                             channel_multiplier=0,
                             allow_small_or_imprecise_dtypes=True)
              irow = const.tile([P, P], f32, name="irow")
              nc.gpsimd.iota(irow[:], pattern=[[1, P]], base=0,
                             channel_multiplier=0,
                             allow_small_or_imprecise_dtypes=True)
              icol = const.tile([P, 1], f32, name="icol")
              nc.gpsimd.iota(icol[:], pattern=[[0, 1]], base=0,
                             channel_multiplier=1,
                             allow_small_or_imprecise_dtypes=True)
              ident = const.tile([P, P], f32, name="ident")
              nc.vector.tensor_scalar(out=ident[:], in0=irow[:],
                                      scalar1=icol[:], scalar2=None,
                                      op0=AL.is_equal)
              ones = const.tile([P, 1], f32, name="ones")
              nc.gpsimd.memset(ones[:], 1.0)
              lnb = const.tile([P, 1], f32, name="lnb")
              nc.gpsimd.memset(lnb[:], 1e-30)

              # gt broadcast [P, 160]
              gtb = const.tile([P, 5 * G], f32, name="gtb")
              gsrc = gtaux[:]
              gsrc_b = dataclasses.replace(gsrc, ap=[[0, P]] + gsrc.ap[1:])
              nc.sync.dma_start(gtb[:], gsrc_b)
              # areaB [P, 32]
              wg_r = const.tile([P, G], f32, name="wg_r")
              nc.vector.tensor_tensor(out=wg_r[:], in0=gtb[:, 2 * G:3 * G],
                                      in1=gtb[:, 0:G], op=AL.subtract)
              hg_r = const.tile([P, G], f32, name="hg_r")
              nc.vector.tensor_tensor(out=hg_r[:], in0=gtb[:, 3 * G:4 * G],
                                      in1=gtb[:, G:2 * G], op=AL.subtract)
              areaB = const.tile([P, G], f32, name="areaB")
              nc.vector.tensor_tensor(out=areaB[:], in0=wg_r[:], in1=hg_r[:],
                                      op=AL.mult)

              # gather table, block-diagonal [P, 48]:
              # rows 32fs..32fs+32, cols 12fs..12fs+12 = [xg yg lwg lhg oh8]
              tt16 = const.tile([P, 48], f32, name="tt16")
              nc.gpsimd.memset(tt16[:], 0.0)
              traw = const.tile([G, 5], f32, name="traw")
              gsrc2 = dataclasses.replace(gsrc, ap=[[1, G], [G, 5]])
              nc.sync.dma_start(traw[:], gsrc2)
              tblk = const.tile([G, 12], f32, name="tblk")
              ttmp = const.tile([G, 1], f32, name="ttmp")
              # xg, yg
              nc.vector.tensor_tensor(out=ttmp[:], in0=traw[:, 0:1],
                                      in1=traw[:, 2:3], op=AL.add)
              nc.vector.tensor_scalar(out=tblk[:, 0:1], in0=ttmp[:],
                                      scalar1=0.5, scalar2=None, op0=AL.mult)
              nc.vector.tensor_tensor(out=ttmp[:], in0=traw[:, 1:2],
                                      in1=traw[:, 3:4], op=AL.add)
              nc.vector.tensor_scalar(out=tblk[:, 1:2], in0=ttmp[:],
                                      scalar1=0.5, scalar2=None, op0=AL.mult)
              # ln(wg), ln(hg)
              nc.vector.tensor_tensor(out=ttmp[:], in0=traw[:, 2:3],
                                      in1=traw[:, 0:1], op=AL.subtract)
              nc.scalar.activation(tblk[:, 2:3], ttmp[:], AF.Ln)
              nc.vector.tensor_tensor(out=ttmp[:], in0=traw[:, 3:4],
                                      in1=traw[:, 1:2], op=AL.subtract)
              nc.scalar.activation(tblk[:, 3:4], ttmp[:], AF.Ln)
              # one-hot(label)
              io8 = const.tile([G, C], f32, name="io8")
              nc.gpsimd.iota(io8[:], pattern=[[1, C]], base=0,
                             channel_multiplier=0,
                             allow_small_or_imprecise_dtypes=True)
              nc.vector.tensor_scalar(out=tblk[:, 4:12], in0=io8[:],
                                      scalar1=traw[:, 4:5], scalar2=None,
                                      op0=AL.is_equal)
              for fs in range(4):
                  nc.sync.dma_start(tt16[32 * fs:32 * fs + 32,
                                         12 * fs:12 * fs + 12], tblk[:])

              # ---------- anchors + per-anchor prep ----------
              pA_stack = ExitStack()
              pA = pA_stack.enter_context(tc.tile_pool(name="pA", bufs=1))
              ax1 = pA.tile([P, COLS], f32, name="ax1")
              ay1 = pA.tile([P, COLS], f32, name="ay1")
              ax2 = pA.tile([P, COLS], f32, name="ax2")
              ay2 = pA.tile([P, COLS], f32, name="ay2")
              for t, c in ((ax1, 0), (ay1, 1), (ax2, 2), (ay2, 3)):
                  nc.sync.dma_start(t[:], plane(anch, c))
              xa = persist.tile([P, COLS], f32, name="xa")
              ya = persist.tile([P, COLS], f32, name="ya")
              iwa = persist.tile([P, COLS], f32, name="iwa")
              iha = persist.tile([P, COLS], f32, name="iha")
              La = persist.tile([P, COLS], f32, name="La")
              Ha = persist.tile([P, COLS], f32, name="Ha")
              area_a = pA.tile([P, COLS], f32, name="area_a")
              prep_stack = ExitStack()
              prepp = prep_stack.enter_context(tc.tile_pool(name="prepp", bufs=1))
              wa_e = prepp.tile([P, COLS], f32, name="wa_e")
              ha_e = prepp.tile([P, COLS], f32, name="ha_e")
              nc.vector.scalar_tensor_tensor(out=wa_e[:], in0=ax2[:], scalar=EPS,
                                             in1=ax1[:], op0=AL.add,
                                             op1=AL.subtract)
              nc.vector.scalar_tensor_tensor(out=ha_e[:], in0=ay2[:], scalar=EPS,
                                             in1=ay1[:], op0=AL.add,
                                             op1=AL.subtract)
              nc.vector.scalar_tensor_tensor(out=area_a[:], in0=wa_e[:],
                                             scalar=-EPS, in1=ha_e[:],
                                             op0=AL.add, op1=AL.mult)
              nc.gpsimd.tensor_tensor(out=xa[:], in0=ax1[:], in1=ax2[:], op=AL.add)
              nc.gpsimd.tensor_scalar(out=xa[:], in0=xa[:], scalar1=0.5,
                                      scalar2=None, op0=AL.mult)
              nc.gpsimd.tensor_tensor(out=ya[:], in0=ay1[:], in1=ay2[:], op=AL.add)
              nc.gpsimd.tensor_scalar(out=ya[:], in0=ya[:], scalar1=0.5,
                                      scalar2=None, op0=AL.mult)
              nc.vector.reciprocal(iwa[:], wa_e[:])
              nc.vector.reciprocal(iha[:], ha_e[:])
              nc.scalar.activation(La[:], wa_e[:], AF.Ln)
              nc.scalar.activation(Ha[:], ha_e[:], AF.Ln)
              prep_stack.close()

              maxv = pA.tile([P, COLS], f32, name="maxv")
              bestp = pA.tile([P, COLS], f32, name="bestp")
              nc.vector.memset(maxv[:], -1e30)
              nc.vector.memset(bestp[:], 0.0)

              # ---------- phase A: j-loop, column-split ----------
              # GP cols [0, cg): GP computes inter/denom; ACT takes logs; DVE tail.
              # DVE cols [cg, COLS): full DVE chain with ACT relu.
              dw = COLS - cg
              with tc.tile_pool(name="jtmp", bufs=1) as jt:
                  if dw > 0:
                      d_ltx2 = [jt.tile([P, dw], f32, name=f"d_ltx{i}") for i in range(2)]
                      d_lty2 = [jt.tile([P, dw], f32, name=f"d_lty{i}") for i in range(2)]
                      d_wxr = [jt.tile([P, dw], f32, name=f"d_wxr{i}") for i in range(2)]
                      d_wxp = [jt.tile([P, dw], f32, name=f"d_wxp{i}") for i in range(2)]
                      d_wyr2 = [jt.tile([P, dw], f32, name=f"d_wyr{i}") for i in range(2)]
                      d_rd2 = (None if logchain else
                               [jt.tile([P, dw], f32, name=f"d_rd{i}") for i in range(2)])
                      d_upd2 = [jt.tile([P, dw], f32, name=f"d_upd{i}") for i in range(2)]
                      d_li = [jt.tile([P, dw], f32, name=f"d_li{i}") for i in range(2)]
                      d_ld = [jt.tile([P, dw], f32, name=f"d_ld{i}") for i in range(2)]
                      d_int2 = [jt.tile([P, dw], f32, name=f"d_int{i}") for i in range(3)]
                      d_den2 = [jt.tile([P, dw], f32, name=f"d_den{i}") for i in range(3)]
                  if cg > 0:
                      g_ltx = jt.tile([P, cg], f32, name="g_ltx")
                      g_lty = jt.tile([P, cg], f32, name="g_lty")
                      g_mnx = jt.tile([P, cg], f32, name="g_mnx")
                      g_wxr = [jt.tile([P, cg], f32, name=f"g_wxr{i}") for i in range(2)]
                      g_wxp = [jt.tile([P, cg], f32, name=f"g_wxp{i}") for i in range(2)]
                      g_wyr = [jt.tile([P, cg], f32, name=f"g_wyr{i}") for i in range(2)]
                      g_wyp = [jt.tile([P, cg], f32, name=f"g_wyp{i}") for i in range(2)]
                      g_ss = [jt.tile([P, cg], f32, name=f"g_ss{i}") for i in range(3)]
                      g_int = [jt.tile([P, cg], f32, name=f"g_int{i}") for i in range(3)]
                      g_li = [jt.tile([P, cg], f32, name=f"g_li{i}") for i in range(3)]
                      g_ld = [jt.tile([P, cg], f32, name=f"g_ld{i}") for i in range(3)]
                      g_lio = [jt.tile([P, cg], f32, name=f"g_lio{i}") for i in range(2)]
                      g_upd = jt.tile([P, cg], f32, name="g_upd")

                  def jstep_dve(j):
                      if dw == 0:
                          return
                      gx1 = gtb[:, j:j + 1]
                      gy1 = gtb[:, G + j:G + j + 1]
                      gx2 = gtb[:, 2 * G + j:2 * G + j + 1]
                      gy2 = gtb[:, 3 * G + j:3 * G + j + 1]
                      aB = areaB[:, j:j + 1]
                      s_ = slice(cg, COLS)
                      wxr = d_wxr[j % 2]
                      wxp = d_wxp[j % 2]
                      d_int = d_int2[j % 3]
                      d_den = d_den2[j % 3]
                      d_ltx = d_ltx2[j % 2]
                      d_lty = d_lty2[j % 2]
                      d_wyr = d_wyr2[j % 2]
                      d_rd = d_rd2[j % 2] if d_rd2 is not None else None
                      v = nc.vector
                      v.tensor_scalar(out=d_ltx[:], in0=ax1[:, s_], scalar1=gx1,
                                      scalar2=None, op0=AL.max)
                      v.scalar_tensor_tensor(out=wxr[:], in0=ax2[:, s_], scalar=gx2,
                                             in1=d_ltx[:], op0=AL.min,
                                             op1=AL.subtract)
                      nc.scalar.activation(wxp[:], wxr[:], AF.Relu)
                      v.tensor_scalar(out=d_lty[:], in0=ay1[:, s_], scalar1=gy1,
                                      scalar2=None, op0=AL.max)
                      v.scalar_tensor_tensor(out=d_wyr[:], in0=ay2[:, s_], scalar=gy2,
                                             in1=d_lty[:], op0=AL.min,
                                             op1=AL.subtract)
                      v.scalar_tensor_tensor(out=d_int[:], in0=d_wyr[:],
                                             scalar=0.0, in1=wxp[:],
                                             op0=AL.max, op1=AL.mult)
                      # Ssum = area_a + areaB_j (no inter dependency; on ACT)
                      nc.scalar.activation(d_den[:], area_a[:, s_], AF.Identity,
                                           bias=aB)
                      if logchain:
                          dli = d_li[j % 2]
                          dld = d_ld[j % 2]
                          nc.scalar.activation(dli[:], d_int[:], AF.Ln,
                                               bias=lnb[:])
                          nc.scalar.activation(dld[:], d_den[:], AF.Ln)
                          iou = d_lty  # log(t), t = inter/Ssum (monotone in iou)
                          v.scalar_tensor_tensor(out=iou[:], in0=dld[:],
                                                 scalar=-1.0, in1=dli[:],
                                                 op0=AL.mult, op1=AL.add)
                      else:
                          v.reciprocal(d_rd[:], d_den[:])
                          iou = d_lty  # t = inter/Ssum (monotone in iou)
                          v.tensor_tensor(out=iou[:], in0=d_int[:], in1=d_rd[:],
                                          op=AL.mult)
                      upd = d_upd2[j % 2]
                      v.tensor_tensor(out=upd[:], in0=iou[:], in1=maxv[:, s_],
                                      op=AL.is_gt)
                      v.tensor_tensor(out=maxv[:, s_], in0=maxv[:, s_], in1=iou[:],
                                      op=AL.max)
                      v.scalar_tensor_tensor(out=bestp[:, s_], in0=upd[:],
                                             scalar=float(j + 1), in1=bestp[:, s_],
                                             op0=AL.mult, op1=AL.max)

                  def jstep_gp(j):
                      if cg == 0:
                          return
                      gx1 = gtb[:, j:j + 1]
                      gy1 = gtb[:, G + j:G + j + 1]
                      gx2 = gtb[:, 2 * G + j:2 * G + j + 1]
                      gy2 = gtb[:, 3 * G + j:3 * G + j + 1]
                      aB = areaB[:, j:j + 1]
                      s_ = slice(0, cg)
                      gi_ = g_int[j % 3]
                      li = g_li[j % 3]
                      ld = g_ld[j % 3]
                      wxr = g_wxr[j % 2]; wxp = g_wxp[j % 2]
                      wyr = g_wyr[j % 2]; wyp = g_wyp[j % 2]
                      ss = g_ss[j % 3]
                      g = nc.gpsimd
                      g.tensor_scalar(out=g_ltx[:], in0=ax1[:, s_], scalar1=gx1,
                                      scalar2=None, op0=AL.max)
                      g.tensor_scalar(out=g_mnx[:], in0=ax2[:, s_], scalar1=gx2,
                                      scalar2=None, op0=AL.min)
                      g.tensor_tensor(out=wxr[:], in0=g_mnx[:], in1=g_ltx[:],
                                      op=AL.subtract)
                      nc.scalar.activation(wxp[:], wxr[:], AF.Relu)
                      g.tensor_scalar(out=g_lty[:], in0=ay1[:, s_], scalar1=gy1,
                                      scalar2=None, op0=AL.max)
                      g.tensor_scalar(out=g_mnx[:], in0=ay2[:, s_], scalar1=gy2,
                                      scalar2=None, op0=AL.min)
                      g.tensor_tensor(out=wyr[:], in0=g_mnx[:], in1=g_lty[:],
                                      op=AL.subtract)
                      nc.scalar.activation(wyp[:], wyr[:], AF.Relu)
                      g.tensor_tensor(out=gi_[:], in0=wxp[:], in1=wyp[:],
                                      op=AL.mult)
                      nc.scalar.activation(ss[:], area_a[:, s_], AF.Identity,
                                           bias=aB)
                      # logs on ACT; compare in log-t space (t = inter/Ssum)
                      nc.scalar.activation(li[:], gi_[:], AF.Ln, bias=lnb[:])
                      nc.scalar.activation(ld[:], ss[:], AF.Ln)

                  def jtail_gp(j):
                      if cg == 0:
                          return
                      s_ = slice(0, cg)
                      li = g_li[j % 3]
                      ld = g_ld[j % 3]
                      lio = g_lio[j % 2]
                      v = nc.vector
                      v.scalar_tensor_tensor(out=lio[:], in0=ld[:], scalar=-1.0,
                                             in1=li[:], op0=AL.mult, op1=AL.add)
                      v.tensor_tensor(out=g_upd[:], in0=lio[:], in1=maxv[:, s_],
                                      op=AL.is_gt)
                      v.tensor_tensor(out=maxv[:, s_], in0=maxv[:, s_],
                                      in1=lio[:], op=AL.max)
                      v.scalar_tensor_tensor(out=bestp[:, s_], in0=g_upd[:],
                                             scalar=float(j + 1), in1=bestp[:, s_],
                                             op0=AL.mult, op1=AL.max)

                  DELAY = 2
                  for j in range(G):
                      jstep_gp(j)
                      jstep_dve(j)
                      if j >= DELAY:
                          jtail_gp(j - DELAY)
                  for j in range(G - DELAY, G):
                      jtail_gp(j)

              # pos & masked bestp (padded to W); GP cols compare in log domain
              nposA = persist.tile([P, 1], f32, name="nposA")
              sl1A = persist.tile([P, 1], f32, name="sl1A")
              nsumA = persist.tile([P, 1], f32, name="nsumA")
              corrA = persist.tile([P, 1], f32, name="corrA")
              tacc = persist.tile([P, 1], f32, name="tacc")
              for t in (nposA, sl1A, nsumA, corrA):
                  nc.vector.memset(t[:], 0.0)
              pos = persist.tile([P, COLS], f32, name="pos")
              if cg > 0:
                  nc.vector.tensor_scalar(out=pos[:, 0:cg], in0=maxv[:, 0:cg],
                                          scalar1=LN_THIRD, scalar2=None,
                                          op0=AL.is_ge, op1=AL.add,
                                          accum_out=tacc[:])
                  nc.vector.tensor_tensor(out=nposA[:], in0=nposA[:],
                                          in1=tacc[:], op=AL.add)
              if COLS > cg:
                  thr = LN_THIRD if logchain else THIRD
                  nc.vector.tensor_scalar(out=pos[:, cg:COLS],
                                          in0=maxv[:, cg:COLS],
                                          scalar1=thr, scalar2=None,
                                          op0=AL.is_ge, op1=AL.add,
                                          accum_out=tacc[:])
                  nc.vector.tensor_tensor(out=nposA[:], in0=nposA[:],
                                          in1=tacc[:], op=AL.add)
              bpm = persist.tile([P, W], f32, name="bpm")
              nc.vector.memset(bpm[:], 0.0)
              nc.vector.tensor_tensor(out=bpm[:, 0:COLS], in0=pos[:],
                                      in1=bestp[:], op=AL.mult)
              pA_stack.close()

              # ---------- phase B ----------
              with ExitStack() as bctx:
                  ohp = bctx.enter_context(tc.tile_pool(name="ohp", bufs=2))
                  psum_t = bctx.enter_context(
                      tc.tile_pool(name="psum_t", bufs=2, space="PSUM"))
                  psum_g = bctx.enter_context(
                      tc.tile_pool(name="psum_g", bufs=2, space="PSUM"))
                  gath_p = bctx.enter_context(tc.tile_pool(name="gath", bufs=2))
                  scr = bctx.enter_context(tc.tile_pool(name="scr", bufs=1))
                  dmap = bctx.enter_context(tc.tile_pool(name="dmap", bufs=3))

                  sA1 = scr.tile([P, WH], f32, name="sA1")
                  sA2 = scr.tile([P, WH], f32, name="sA2")
                  sA3 = scr.tile([P, WH], f32, name="sA3")
                  sA4 = scr.tile([P, WH], f32, name="sA4")
                  sA5 = scr.tile([P, WH], f32, name="sA5")
                  sA6 = scr.tile([P, WH], f32, name="sA6")
                  sB1 = scr.tile([P, WH], f32, name="sB1")
                  sB2 = scr.tile([P, WH], f32, name="sB2")
                  sB3 = scr.tile([P, WH], f32, name="sB3")
                  sB4 = scr.tile([P, WH], f32, name="sB4")
                  sB5 = scr.tile([P, WH], f32, name="sB5")
                  sB6 = scr.tile([P, WH], f32, name="sB6")
                  s5 = sA5
                  fE = [scr.tile([P, WH], f32, name=f"fE{i}") for i in range(2)]
                  fU = [scr.tile([P, WH], f32, name=f"fU{i}") for i in range(2)]
                  fS = [scr.tile([P, WH], f32, name=f"fS{i}") for i in range(2)]
                  fG = [scr.tile([P, WH], f32, name=f"fG{i}") for i in range(2)]
                  fN = [scr.tile([P, WH], f32, name=f"fN{i}") for i in range(2)]
                  fP = [scr.tile([P, WH], f32, name=f"fP{i}") for i in range(2)]
                  Rp = [scr.tile([P, WH], f32, name=f"Rp{c}") for c in range(C)]

                  for half in range(2):
                      base = half * WH
                      rw = min(COLS - base, WH)   # real (unpadded) width
                      if rw <= 0:
                          break
                      gath = gath_p.tile([P, 12 * WH], f32, name="gath")

                      def gpl(m):
                          return gath[:, m * WH:m * WH + rw]

                      # gather: superquads
                      for s in range(HS):
                          sq = half * HS + s
                          oh = ohp.tile([P, 512], f32, name="oh")
                          src = _expand32(bpm[:, 16 * sq:16 * sq + 16])
                          nc.vector.tensor_tensor(
                              out=oh[:].rearrange("p (f j) -> p f j", j=G),
                              in0=src,
                              in1=iotarep[:].rearrange("p (f j) -> p f j", j=G),
                              op=AL.is_equal)
                          pt = psum_t.tile([P, 512], f32, name="pt")
                          for t4 in range(4):
                              nc.tensor.transpose(pt[:, 128 * t4:128 * t4 + 128],
                                                  oh[:, 128 * t4:128 * t4 + 128],
                                                  ident[:])
                          ohT = ohp.tile([P, 512], f32, name="ohT")
                          if s % 2 == 0:
                              nc.scalar.copy(ohT[:], pt[:])
                          else:
                              nc.vector.tensor_copy(ohT[:], pt[:])
                          gp = psum_g.tile([P, 192], f32, name="gp")
                          for t4 in range(4):
                              nc.tensor.matmul(out=gp[:, 48 * t4:48 * t4 + 48],
                                               lhsT=ohT[:, 128 * t4:128 * t4 + 128],
                                               rhs=tt16[:], start=True, stop=True)
                          # scatter copy psum -> planar gath slices
                          src_g = gp[:].rearrange("p (t f m) -> p t f m", t=4, f=4)
                          dst = gath[:]
                          dst_ap = dataclasses.replace(
                              dst, offset=dst.offset + 16 * s,
                              ap=[dst.ap[0], [4, 4], [1, 4], [WH, 12]])
                          nc.scalar.copy(dst_ap, src_g)

                      posh = pos[:, base:base + rw]
                      xah = xa[:, base:base + rw]
                      yah = ya[:, base:base + rw]
                      iwah = iwa[:, base:base + rw]
                      ihah = iha[:, base:base + rw]
                      Lah = La[:, base:base + rw]
                      Hah = Ha[:, base:base + rw]

                      # ---- reg: targets + smooth-L1 ----
                      for k, (gm, ctr, inv, lg) in enumerate(
                              ((0, xah, iwah, None), (1, yah, ihah, None),
                               (2, None, None, Lah), (3, None, None, Hah))):
                          s1, s2_, s3 = (sA1, sA2, sA3) if k % 2 == 0 else (sB1, sB2, sB3)
                          s4, s5, s6 = (sA4, sA5, sA6) if k % 2 == 0 else (sB4, sB5, sB6)
                          rt = s1
                          if lg is None:
                              nc.vector.tensor_tensor(out=s2_[:, :rw], in0=gpl(gm),
                                                      in1=ctr, op=AL.subtract)
                              nc.vector.tensor_tensor(out=rt[:, :rw], in0=s2_[:, :rw],
                                                      in1=inv, op=AL.mult)
                          else:
                              nc.vector.tensor_tensor(out=rt[:, :rw], in0=gpl(gm),
                                                      in1=lg, op=AL.subtract)
                          rp = dmap.tile([P, WH], f32, name="rp")
                          rsrc = plane(regp, k)
                          rsl = dataclasses.replace(
                              rsrc, offset=rsrc.offset + base,
                              ap=[rsrc.ap[0], [1, rw]])
                          nc.sync.dma_start(rp[:, :rw], rsl)
                          e = s2_
                          nc.vector.tensor_tensor(out=e[:, :rw], in0=rp[:, :rw],
                                                  in1=rt[:, :rw], op=AL.subtract)
                          q = s3
                          nc.scalar.activation(q[:, :rw], e[:, :rw], AF.Abs)
                          qm = s4
                          nc.gpsimd.tensor_tensor(out=qm[:, :rw], in0=q[:, :rw],
                                                  in1=posh, op=AL.mult)
                          cm = s5
                          nc.gpsimd.tensor_scalar(out=cm[:, :rw], in0=qm[:, :rw],
                                                  scalar1=BETA, scalar2=None,
                                                  op0=AL.min)
                          t2 = s6
                          nc.vector.scalar_tensor_tensor(out=t2[:, :rw],
                                                         in0=qm[:, :rw],
                                                         scalar=2.0,
                                                         in1=cm[:, :rw],
                                                         op0=AL.mult,
                                                         op1=AL.subtract)
                          nc.vector.scalar_tensor_tensor(out=e[:, :rw],
                                                         in0=cm[:, :rw],
                                                         scalar=0.0,
                                                         in1=t2[:, :rw],
                                                         op0=AL.add, op1=AL.mult,
                                                         accum_out=tacc[:])
                          nc.vector.tensor_tensor(out=sl1A[:], in0=sl1A[:],
                                                  in1=tacc[:], op=AL.add)

                      # ---- focal (gather-independent part) ----
                      for c in range(C):
                          xc = dmap.tile([P, WH], f32, name="xc")
                          csrc = plane(clsp, c)
                          csl = dataclasses.replace(
                              csrc, offset=csrc.offset + base,
                              ap=[csrc.ap[0], [1, rw]])
                          nc.sync.dma_start(xc[:, :rw], csl)
                          E = fE[c % 2]
                          nc.scalar.activation(E[:, :rw], xc[:, :rw], AF.Exp,
                                               scale=-1.0)
                          u = fU[c % 2]
                          nc.gpsimd.tensor_scalar(out=u[:, :rw], in0=E[:, :rw],
                                                  scalar1=1.0, scalar2=None,
                                                  op0=AL.add)
                          spn = fS[c % 2]
                          nc.scalar.activation(spn[:, :rw], u[:, :rw], AF.Ln)
                          sig = fG[c % 2]
                          nc.vector.reciprocal(sig[:, :rw], u[:, :rw])
                          sgn = fN[c % 2]
                          nc.gpsimd.tensor_tensor(out=sgn[:, :rw], in0=E[:, :rw],
                                                  in1=sig[:, :rw], op=AL.mult)
                          sp = fP[c % 2]
                          nc.gpsimd.tensor_tensor(out=sp[:, :rw], in0=xc[:, :rw],
                                                  in1=spn[:, :rw], op=AL.add)
                          s2t = E  # reuse: sig^2
                          nc.scalar.activation(s2t[:, :rw], sig[:, :rw], AF.Square)
                          Nt = sig  # N = sig^2 * sp  (overwrite sig)
                          nc.vector.scalar_tensor_tensor(out=Nt[:, :rw],
                                                         in0=s2t[:, :rw],
                                                         scalar=0.0,
                                                         in1=sp[:, :rw],
                                                         op0=AL.add, op1=AL.mult,
                                                         accum_out=tacc[:])
                          nc.vector.tensor_tensor(out=nsumA[:], in0=nsumA[:],
                                                  in1=tacc[:], op=AL.add)
                          s2n = sp  # reuse: sgn^2
                          nc.gpsimd.tensor_tensor(out=s2n[:, :rw], in0=sgn[:, :rw],
                                                  in1=sgn[:, :rw], op=AL.mult)
                          Pt = fU[c % 2]  # P = sgn^2 * spn
                          nc.gpsimd.tensor_tensor(out=Pt[:, :rw], in0=s2n[:, :rw],
                                                  in1=spn[:, :rw], op=AL.mult)
                          nc.vector.scalar_tensor_tensor(out=Rp[c][:, :rw],
                                                         in0=Pt[:, :rw],
                                                         scalar=1.0 / 3.0,
                                                         in1=Nt[:, :rw],
                                                         op0=AL.mult,
                                                         op1=AL.subtract)
                      # ---- corr dots (need gather) ----
                      for c in range(C):
                          s5c = sA5 if c % 2 == 0 else sB5
                          nc.gpsimd.tensor_tensor(out=s5c[:, :rw],
                                                  in0=gpl(4 + c),
                                                  in1=Rp[c][:, :rw], op=AL.mult)
                          nc.scalar.activation(s5c[:, :rw], s5c[:, :rw],
                                               AF.Identity, accum_out=tacc[:])
                          nc.vector.tensor_tensor(out=corrA[:], in0=corrA[:],
                                                  in1=tacc[:], op=AL.add)

              # ---------- final cross-partition reduce ----------
              acc4 = persist.tile([P, 4], f32, name="acc4")
              nc.scalar.copy(acc4[:, 0:1], nposA[:])
              nc.scalar.copy(acc4[:, 1:2], sl1A[:])
              nc.scalar.copy(acc4[:, 2:3], nsumA[:])
              nc.scalar.copy(acc4[:, 3:4], corrA[:])
              with tc.tile_pool(name="psum_f", bufs=1, space="PSUM") as pf:
                  fps = pf.tile([1, 4], f32, name="fps")
                  nc.tensor.matmul(out=fps[:], lhsT=ones[:], rhs=acc4[:],
                                   start=True, stop=True)
                  osb = persist.tile([1, 4], f32, name="osb")
                  nc.scalar.copy(osb[:], fps[:])
                  nc.sync.dma_start(out[:], osb[:])

    return nc


# ---------------- host side ----------------

def pack_inputs(cls_preds, reg_preds, anchors, gt_boxes, gt_labels):
    """Full inputs -> list of 8 per-core input maps (planar layouts)."""
    B, A, _ = cls_preds.shape
    anch = np.ascontiguousarray(anchors.astype(np.float32).T)         # [4, A]
    maps = []
    for b in range(B):
        clsp = np.ascontiguousarray(cls_preds[b].astype(np.float32).T)  # [8, A]
        regp = np.ascontiguousarray(reg_preds[b].astype(np.float32).T)  # [4, A]
        gb = gt_boxes[b].astype(np.float32)
        lab = gt_labels[b].astype(np.float32)
        gtaux = np.concatenate([gb[:, 0], gb[:, 1], gb[:, 2], gb[:, 3],
                                lab]).astype(np.float32)[None, :]       # [1,160]
        maps.append({"anch": anch, "clsp": clsp, "regp": regp,
                     "gtaux": gtaux})
    return maps


def finish(partials):
    """partials: list of [1,4] arrays per core -> (cls_loss, reg_loss)."""
    f = np.float32
    npos = f(0); sl1 = f(0); nsum = f(0); corr = f(0)
    for p in partials:
        p = p.reshape(4)
        npos += f(p[0]); sl1 += f(p[1]); nsum += f(p[2]); corr += f(p[3])
    denom = max(float(npos), 1.0)
    if npos > 0:
        cls_loss = f(0.75) * (nsum + corr) / f(denom)
        reg_loss = sl1 / f(2 * BETA) / f(denom)
    else:
        cls_loss = f(0.0); reg_loss = f(0.0)
    return np.float32(cls_loss), np.float32(reg_loss)


# ---------------- self-contained kernel entry ----------------

_CACHE = {}

def _get_fn(n_cores=8):
    """Build + jit the 8-core SPMD executable once."""
    if "fn" in _CACHE:
        return _CACHE["fn"]
    import jax
    from jax.sharding import Mesh, PartitionSpec, NamedSharding
    from jax.experimental.shard_map import shard_map
    from concourse.bass2jax import (_bass_exec_p, install_neuronx_cc_hook,
                                    partition_id_tensor)
    patch_tile_drain(1)
    nc = build(160000, cg=512, logchain=True)
    split_sync_waits(nc)
    install_neuronx_cc_hook()
    in_names, out_names, out_avals, zero_shapes = [], [], [], []
    partition_name = (nc.partition_id_tensor.name
                      if nc.partition_id_tensor else None)
    for alloc in nc.m.functions[0].allocations:
        if not isinstance(alloc, mybir.MemoryLocationSet):
            continue
        name = alloc.memorylocations[0].name
        if alloc.kind == "ExternalInput":
            if name != partition_name:
                in_names.append(name)
        elif alloc.kind == "ExternalOutput":
            out_names.append(name)
            shape = tuple(alloc.tensor_shape)
            dtype = mybir.dt.np(alloc.dtype)
            out_avals.append(jax.core.ShapedArray(shape, dtype))
            zero_shapes.append((shape, dtype))
    n_params = len(in_names)
    n_outs = len(out_avals)
    all_in_names = in_names + out_names + ([partition_name]
                                           if partition_name else [])
    donate = tuple(range(n_params, n_params + n_outs))

    def _body(*args):
        operands = list(args)
        if partition_name is not None:
            operands.append(partition_id_tensor())
        outs = _bass_exec_p.bind(
            *operands, out_avals=tuple(out_avals),
            in_names=tuple(all_in_names), out_names=tuple(out_names),
            lowering_input_output_aliases=(),
            sim_require_finite=True, sim_require_nnan=True, nc=nc)
        return tuple(outs)

    devices = jax.devices()[:n_cores]
    mesh = Mesh(np.asarray(devices), ("core",))
    in_specs = (PartitionSpec("core"),) * (n_params + n_outs)
    out_specs = (PartitionSpec("core"),) * len(out_names)
    fn = jax.jit(shard_map(_body, mesh=mesh, in_specs=in_specs,
                           out_specs=out_specs, check_rep=False),
                 donate_argnums=donate, keep_unused=True)
    sh = NamedSharding(mesh, PartitionSpec("core"))
    _CACHE["fn"] = (fn, in_names, out_names, out_avals, zero_shapes, sh,
                    n_cores)
    return _CACHE["fn"]


def kernel(cls_preds, reg_preds, anchors, gt_boxes, gt_labels):
    """Full-input DetectionLoss on 8 NeuronCores (data-parallel over batch).

    Returns (cls_loss, reg_loss) as float32 scalars, matching reference()."""
    import jax
    cls_preds = np.asarray(cls_preds)
    reg_preds = np.asarray(reg_preds)
    anchors = np.asarray(anchors)
    gt_boxes = np.asarray(gt_boxes)
    gt_labels = np.asarray(gt_labels)
    B, A, _ = cls_preds.shape
    assert (B, A) == (8, 160000), (B, A)
    maps = pack_inputs(cls_preds, reg_preds, anchors, gt_boxes, gt_labels)
    fn, in_names, out_names, out_avals, zero_shapes, sh, n_cores = _get_fn()
    concat_in = [jax.device_put(
        np.concatenate([np.asarray(maps[c][nm]) for c in range(n_cores)],
                       axis=0), sh) for nm in in_names]
    zeros = [jax.device_put(
        np.zeros((n_cores * s[0], *s[1:]), d), sh) for s, d in zero_shapes]
    out_arrs = fn(*concat_in, *zeros)
    res = np.asarray(out_arrs[out_names.index("out")]).reshape(n_cores, 1, 4)
    partials = [res[c] for c in range(n_cores)]
    cls_loss, reg_loss = finish(partials)
    return cls_loss, reg_loss



# revision 30
# speedup vs baseline: 1.6699x; 1.6699x over previous
"""DetectionLoss Bass/Tile kernel for TRN2 (one core = one image; SPMD x8).

v2: fp16 geometry + log-domain IoU compare + eps-packed argmax.

Per core (image b), layout anchor a <-> (partition p = a // COLS, col a % COLS):
  Phase A (j-loop over G=32 gts, column-split DVE/GPSIMD, ACT relu+ln):
    maxp[a] = max_j [ ln(inter_scaled) - ln(Sa+Sb_j) - j*EPSJ ]
    (monotone in t = inter/(Sa+Sb); argmax j recovered from the eps term.)
  Decode: pos = maxp >= ln(1/3) + LNSC; bestp1 = argmax j + 1; bpm = pos*bestp1.
  Gather (PE): one-hot(bpm) per level -> transpose -> matmul vs [128,20] table
    => per-anchor [xg2, yg2, lwg, lhg, lab+1] (fp16).
  Focal via ACT Softplus/Sigmoid/Square; smooth-L1 on fp16 targets.
  Output [1,6]: npos, sl1_sum, nsum, cp, cn, 0; host finishes.

Host-packed per-core inputs:
  anch16 [10, A] f16 (ax1 ay1 ax2 ay2 xa2 ya2 iwa2 iha2 La Ha)
  area   [1, A]  f32
  clsp   [8, A]  f16, regp [4, A] f16
  gtbs   [1, 160] f32 (gx1 gy1 gx2 gy2 areaB, each [32])
  tbl    [128, 20] f16 (PE gather table, row 4g+c -> cols[5c:5c+5])
"""
import dataclasses
import math
import numpy as np

import concourse.bass as bass
import concourse.mybir as mybir
from concourse import tile

AL = mybir.AluOpType
AF = mybir.ActivationFunctionType
f32 = mybir.dt.float32
f16 = mybir.dt.float16

P = 128
G = 32
C = 8
EPS = 1e-7
BETA = 1.0 / 9.0
EPSJ = 2.0 ** -18
SC = 1.0 / 16.0                      # overlap width prescale (avoids f16 ovf)
LN13 = float(np.float32(math.log(1.0 / 3.0) + 2.0 * math.log(SC)))


def patch_tile_drain(maxw: int = 1):
    """Split the TileContext exit drain's sem waits across NOPs (walrus
    setupSyncWait rejects >1 wait on a CTRL instruction in this build)."""
    import concourse.tile as tile_mod
    from concourse.vector_clock import ScopedClock

    def _drain_and_barrier(self, tick_clock, wait_clock):
        drain_inst = self.nc.sync.drain()
        wait_clock.add_sem_waits(
            drain_inst.ins, ScopedClock({None: tick_clock.global_clock})
        )
        si = drain_inst.ins.sync_info
        waits = list(si.on_wait)
        if len(waits) > maxw:
            si.on_wait = waits[:maxw]
            rest = waits[maxw:]
            for i in range(0, len(rest), maxw):
                nop = self.nc.sync.nop(nofuse=True, hint="drain_split")
                nop.ins.sync_info = mybir.SyncInfo(
                    on_wait=rest[i:i + maxw], on_update=[]
                )
        self.nc.all_engine_barrier()
        assert self.sems is not None
        popped = self.nc._tile_sem_poison_stack.pop()
        assert popped is self._sem_poison
        self.nc.clear_and_free_semaphores(list(self.sems.allocated().values()))
        self.nc.all_engine_barrier()

    tile_mod.TileContext._drain_and_barrier = _drain_and_barrier


def split_sync_waits(nc, maxw: int = 1):
    """Walrus rejects >2 sem waits on one instruction (and >1 on CTRL-type).
    Hoist excess waits onto same-engine NOPs inserted immediately before."""
    ctr = [0]

    def mknop(engine, waits):
        ctr[0] += 1
        nop = mybir.InstNoOp(name=f"I-wsplit-{ctr[0]}", ins=[], outs=[])
        nop.engine = engine
        nop.sync_info = mybir.SyncInfo(on_wait=waits, on_update=[])
        return nop

    for blk in nc.bb_map.values():
        bb = blk.bb
        il = bb.instructions
        i = 0
        while i < len(il):
            inst = il[i]
            si = inst.sync_info
            mw = 1 if isinstance(inst, mybir.InstTensorScalarPtr) else maxw
            if si is not None and len(si.on_wait) > mw:
                waits = list(si.on_wait)
                si.on_wait = waits[:mw]
                rest = waits[mw:]
                for k in range(0, len(rest), 1):
                    il.insert(i, mknop(inst.engine, rest[k:k + 1]))
                    i += 1
            i += 1


def build(A: int, cg: int = 170, gp_focal: int = 2, gp_reg: int = 0,
          gp_oh: int = 0,
          phases: int = 99, gjs: int = 32):
    """Emit the per-core program. A must be divisible by 128.
    cg: GPSIMD stream handles anchor-columns [0, cg); DVE the rest.
    gp_focal: focal classes whose b/Pt muls run on GPSIMD.
    gp_reg: reg planes whose smooth-L1 tail runs on GPSIMD."""
    assert A % P == 0
    COLS = A // P
    W = ((COLS + 63) // 64) * 64
    WH = W // 2
    cg = min(cg, COLS)

    nc = bass.Bass()
    anch = nc.declare_dram_parameter("anch", [10, A], f16, isOutput=False)
    area_d = nc.declare_dram_parameter("area", [1, A], f32, isOutput=False)
    clspd = nc.declare_dram_parameter("clsp", [C, A], f16, isOutput=False)
    regpd = nc.declare_dram_parameter("regp", [4, A], f16, isOutput=False)
    gtbs = nc.declare_dram_parameter("gtbs", [1, 5 * G], f32, isOutput=False)
    tbld = nc.declare_dram_parameter("tbl", [P, 20], f16, isOutput=False)
    out = nc.declare_dram_parameter("out", [1, 6], f32, isOutput=True)

    def plane(t, c):
        return t[c].rearrange("(p w) -> p w", p=P)

    sG = slice(0, cg)
    sD = slice(cg, COLS)

    with tile.TileContext(nc) as tc:
        from contextlib import ExitStack
        with ExitStack() as ctx:
            const = ctx.enter_context(tc.tile_pool(name="const", bufs=1))
            persist = ctx.enter_context(tc.tile_pool(name="persist", bufs=1))

            # ---------- constants ----------
            gtb = const.tile([P, 5 * G], f32, name="gtb")
            gsrc = gtbs[:]
            gsrc_b = dataclasses.replace(gsrc, ap=[[0, P]] + gsrc.ap[1:])
            nc.sync.dma_start(gtb[:], gsrc_b)
            tt20 = const.tile([P, 20], f16, name="tt20")
            nc.sync.dma_start(tt20[:], tbld[:])
            irow = const.tile([P, P], f32, name="irow")
            nc.gpsimd.iota(irow[:], pattern=[[1, P]], base=0,
                           channel_multiplier=0,
                           allow_small_or_imprecise_dtypes=True)
            icol = const.tile([P, 1], f32, name="icol")
            nc.gpsimd.iota(icol[:], pattern=[[0, 1]], base=0,
                           channel_multiplier=1,
                           allow_small_or_imprecise_dtypes=True)
            ident = const.tile([P, P], f16, name="ident")
            nc.vector.tensor_scalar(out=ident[:], in0=irow[:],
                                    scalar1=icol[:], scalar2=None,
                                    op0=AL.is_equal)
            ones = const.tile([P, 1], f32, name="ones")
            nc.gpsimd.memset(ones[:], 1.0)
            lnb = const.tile([P, 1], f32, name="lnb")
            nc.gpsimd.memset(lnb[:], 1e-30)
            scc = const.tile([P, 1], f32, name="scc")
            nc.gpsimd.memset(scc[:], SC)
            nsc = const.tile([P, 1], f32, name="nsc")
            nc.gpsimd.memset(nsc[:], -1.0)
            scsq = const.tile([P, 1], f32, name="scsq")
            nc.gpsimd.memset(scsq[:], SC * SC)

            # ---------- anchor planes ----------
            pa_stack = ExitStack()
            paanch = pa_stack.enter_context(tc.tile_pool(name="paanch", bufs=1))
            ax1 = paanch.tile([P, COLS], f16, name="ax1")
            ay1 = paanch.tile([P, COLS], f16, name="ay1")
            ax2 = paanch.tile([P, COLS], f16, name="ax2")
            ay2 = paanch.tile([P, COLS], f16, name="ay2")
            xa2 = persist.tile([P, COLS], f16, name="xa2")
            ya2 = persist.tile([P, COLS], f16, name="ya2")
            iwa2 = persist.tile([P, COLS], f16, name="iwa2")
            iha2 = persist.tile([P, COLS], f16, name="iha2")
            La = persist.tile([P, COLS], f16, name="La")
            Ha = persist.tile([P, COLS], f16, name="Ha")
            for t, c in ((ax1, 0), (ay1, 1), (ax2, 2), (ay2, 3), (xa2, 4),
                         (ya2, 5), (iwa2, 6), (iha2, 7), (La, 8), (Ha, 9)):
                nc.sync.dma_start(t[:], plane(anch, c))
            area = paanch.tile([P, COLS], f32, name="area")
            nc.sync.dma_start(area[:], plane(area_d, 0))
            # preds loaded early for DMA overlap
            xcs = []
            for c in range(C):
                xc = persist.tile([P, COLS], f16, name=f"xc{c}")
                nc.sync.dma_start(xc[:], plane(clspd, c))
                xcs.append(xc)
            rps = []
            for k in range(4):
                rp = persist.tile([P, COLS], f16, name=f"rp{k}")
                nc.sync.dma_start(rp[:], plane(regpd, k))
                rps.append(rp)

            maxp = persist.tile([P, COLS], f32, name="maxp")
            nc.vector.memset(maxp[:], -1e30)
            maxp2 = persist.tile([P, COLS], f32, name="maxp2")
            nc.vector.memset(maxp2[:], -1e30)

            # ---------- phase A: j-loop ----------
            WD = COLS - cg
            with tc.tile_pool(name="jt", bufs=1) as jt:
                d_nltx = [jt.tile([P, WD], f16, name=f"d_nltx{i}") for i in range(2)]
                d_mnx = [jt.tile([P, WD], f16, name=f"d_mnx{i}") for i in range(2)]
                d_wx = [jt.tile([P, WD], f16, name=f"d_wx{i}") for i in range(2)]
                d_nlty = [jt.tile([P, WD], f16, name=f"d_nlty{i}") for i in range(2)]
                d_mny = [jt.tile([P, WD], f16, name=f"d_mny{i}") for i in range(2)]
                d_wy = [jt.tile([P, WD], f16, name=f"d_wy{i}") for i in range(2)]
                d_wxp = [jt.tile([P, WD], f16, name=f"d_wxp{i}") for i in range(2)]
                d_wyp = [jt.tile([P, WD], f16, name=f"d_wyp{i}") for i in range(2)]
                g_nltx = [jt.tile([P, cg], f16, name=f"g_nltx{i}") for i in range(2)]
                g_wx = [jt.tile([P, cg], f16, name=f"g_wx{i}") for i in range(2)]
                g_nlty = [jt.tile([P, cg], f16, name=f"g_nlty{i}") for i in range(2)]
                g_wy = [jt.tile([P, cg], f16, name=f"g_wy{i}") for i in range(2)]
                g_wyp = [jt.tile([P, cg], f16, name=f"g_wyp{i}") for i in range(2)]
                g_wxp = [jt.tile([P, cg], f16, name=f"g_wxp{i}") for i in range(2)]
                inter = [jt.tile([P, COLS], f16, name=f"inter{i}") for i in range(3)]
                li = [jt.tile([P, COLS], f16, name=f"li{i}") for i in range(2)]
                ld = [jt.tile([P, COLS], f16, name=f"ld{i}") for i in range(2)]
                lt = [jt.tile([P, COLS], f16, name=f"lt{i}") for i in range(2)]
                
                def geom(j):
                    gx1 = gtb[:, j:j + 1]
                    gy1 = gtb[:, G + j:G + j + 1]
                    gx2 = gtb[:, 2 * G + j:2 * G + j + 1]
                    gy2 = gtb[:, 3 * G + j:3 * G + j + 1]
                    aB = gtb[:, 4 * G + j:4 * G + j + 1]
                    it = inter[j % 3]
                    lij = li[j % 2]
                    ldj = ld[j % 2]
                    ltj = lt[j % 2]
                    v = nc.vector
                    g = nc.gpsimd
                    a = nc.scalar
                    # --- GP stream (cols [0, cg)); Pool allows only
                    # 1-op tensor_scalar and tensor_tensor ---
                    if cg > 0:
                        nx = g_nltx[j % 2]; wxg = g_wx[j % 2]
                        ny = g_nlty[j % 2]; wyg = g_wy[j % 2]
                        wpg = g_wyp[j % 2]; wpx = g_wxp[j % 2]
                        g.tensor_scalar(out=nx[:], in0=ax1[:, sG], scalar1=gx1,
                                        scalar2=None, op0=AL.max)
                        g.tensor_scalar(out=wxg[:], in0=ax2[:, sG], scalar1=gx2,
                                        scalar2=None, op0=AL.min)
                        g.tensor_tensor(out=wxg[:], in0=wxg[:], in1=nx[:],
                                        op=AL.subtract)
                        g.tensor_scalar(out=wpx[:], in0=wxg[:], scalar1=0.0,
                                        scalar2=None, op0=AL.max)
                        g.tensor_scalar(out=ny[:], in0=ay1[:, sG], scalar1=gy1,
                                        scalar2=None, op0=AL.max)
                        g.tensor_scalar(out=wyg[:], in0=ay2[:, sG], scalar1=gy2,
                                        scalar2=None, op0=AL.min)
                        g.tensor_tensor(out=wyg[:], in0=wyg[:], in1=ny[:],
                                        op=AL.subtract)
                        g.tensor_scalar(out=wpg[:], in0=wyg[:], scalar1=0.0,
                                        scalar2=None, op0=AL.max)
                        g.tensor_tensor(out=it[:, sG], in0=wpx[:], in1=wpg[:],
                                        op=AL.mult)
                    # --- DVE stream (cols [cg, COLS)), ts/tt + ACT relu ---
                    if WD > 0:
                        nx = d_nltx[j % 2]; mx = d_mnx[j % 2]; wx = d_wx[j % 2]
                        ny = d_nlty[j % 2]; my = d_mny[j % 2]; wy = d_wy[j % 2]
                        wxp = d_wxp[j % 2]; wyp = d_wyp[j % 2]
                        v.tensor_scalar(out=nx[:], in0=ax1[:, sD], scalar1=gx1,
                                        scalar2=-1.0, op0=AL.max, op1=AL.mult)
                        v.tensor_scalar(out=mx[:], in0=ax2[:, sD], scalar1=gx2,
                                        scalar2=None, op0=AL.min)
                        v.tensor_tensor(out=wx[:], in0=mx[:], in1=nx[:],
                                        op=AL.add)
                        a.activation(wxp[:], wx[:], AF.Relu, scale=scc[:])
                        v.tensor_scalar(out=ny[:], in0=ay1[:, sD], scalar1=gy1,
                                        scalar2=-1.0, op0=AL.max, op1=AL.mult)
                        v.tensor_scalar(out=my[:], in0=ay2[:, sD], scalar1=gy2,
                                        scalar2=None, op0=AL.min)
                        v.tensor_tensor(out=wy[:], in0=my[:], in1=ny[:],
                                        op=AL.add)
                        a.activation(wyp[:], wy[:], AF.Relu, scale=scc[:])
                        v.tensor_tensor(out=it[:, sD], in0=wxp[:], in1=wyp[:],
                                        op=AL.mult)
                    # --- ACT logs (ld full width; li per stream) ---
                    a.activation(ldj[:], area[:], AF.Ln, bias=aB)
                    if cg > 0:
                        a.activation(lij[:, sG], it[:, sG], AF.Ln, bias=lnb[:],
                                     scale=scsq[:])
                    if WD > 0:
                        a.activation(lij[:, sD], it[:, sD], AF.Ln, bias=lnb[:])
                def tail(j):
                    ltj = lt[j % 2]
                    lij = li[j % 2]
                    ldj = ld[j % 2]
                    v = nc.vector
                    g = nc.gpsimd
                    mpj = maxp if j % 2 == 0 else maxp2
                    if cg > 0:
                        g.tensor_tensor(out=ltj[:, sG], in0=lij[:, sG],
                                        in1=ldj[:, sG], op=AL.subtract)
                        v.scalar_tensor_tensor(out=mpj[:, sG],
                                               in0=ltj[:, sG],
                                               scalar=-(j * EPSJ),
                                               in1=mpj[:, sG],
                                               op0=AL.add, op1=AL.max)
                    if WD > 0:
                        v.tensor_tensor(out=ltj[:, sD], in0=lij[:, sD],
                                        in1=ldj[:, sD], op=AL.subtract)
                        v.scalar_tensor_tensor(out=mpj[:, sD], in0=ltj[:, sD],
                                               scalar=-(j * EPSJ),
                                               in1=mpj[:, sD],
                                               op0=AL.add, op1=AL.max)

                NJ = min(G, gjs)
                for j in range(NJ):
                    geom(j)
                    if j >= 1:
                        tail(j - 1)
                tail(NJ - 1)

            nc.vector.tensor_tensor(out=maxp[:], in0=maxp[:], in1=maxp2[:],
                                    op=AL.max)
            pa_stack.close()

            # ---------- decode argmax + pos ----------
            nposA = persist.tile([P, 1], f32, name="nposA")
            sl1A = persist.tile([P, 1], f32, name="sl1A")
            nsA = persist.tile([P, 1], f32, name="nsA")
            cpA = persist.tile([P, 1], f32, name="cpA")
            cnA = persist.tile([P, 1], f32, name="cnA")
            for t in (nposA, sl1A, nsA, cpA, cnA):
                nc.vector.memset(t[:], 0.0)
            tac = persist.tile([P, 1], f32, name="tac")
            tac2 = persist.tile([P, 1], f32, name="tac2")

            pos = persist.tile([P, W], f16, name="pos")
            nc.vector.memset(pos[:], 0.0)
            bpm = persist.tile([P, W], f16, name="bpm")
            nc.vector.memset(bpm[:], 0.0)
            with tc.tile_pool(name="dec", bufs=1) as dec:
                mx16 = dec.tile([P, COLS], f16, name="mx16")
                nc.vector.tensor_copy(mx16[:], maxp[:])
                djt = dec.tile([P, COLS], f32, name="djt")
                nc.vector.scalar_tensor_tensor(out=djt[:], in0=maxp[:],
                                               scalar=-1.0, in1=mx16[:],
                                               op0=AL.mult, op1=AL.add)
                bp1 = dec.tile([P, COLS], f16, name="bp1")
                nc.vector.tensor_scalar(out=bp1[:], in0=djt[:],
                                        scalar1=1.0 / EPSJ, scalar2=1537.0,
                                        op0=AL.mult, op1=AL.add)
                bestp1 = dec.tile([P, COLS], f16, name="bestp1")
                nc.vector.tensor_scalar(out=bestp1[:], in0=bp1[:],
                                        scalar1=-1536.0, scalar2=None,
                                        op0=AL.add)
                nc.vector.tensor_scalar(out=pos[:, 0:COLS], in0=maxp[:],
                                        scalar1=LN13, scalar2=None,
                                        op0=AL.is_ge, op1=AL.add,
                                        accum_out=nposA[:])
                nc.vector.tensor_tensor(out=bpm[:, 0:COLS], in0=pos[:, 0:COLS],
                                        in1=bestp1[:], op=AL.mult)

            # ---------- gather via PE ----------
            do_gather = phases >= 2
            do_focal = phases >= 3
            do_reg = phases >= 4
            gath = persist.tile([P, 5 * W], f16, name="gath")

            def gpl(m, lo=0, hi=COLS):
                return gath[:, m * W + lo:m * W + hi]

            with ExitStack() as gctx:
                ohp = gctx.enter_context(tc.tile_pool(name="ohp", bufs=1))
                ptp = gctx.enter_context(
                    tc.tile_pool(name="ptp", bufs=4, space="PSUM"))
                otp = gctx.enter_context(tc.tile_pool(name="otp", bufs=4))
                gpp = gctx.enter_context(
                    tc.tile_pool(name="gpp", bufs=4, space="PSUM"))
                # focal part 1 (gather-independent; overlaps the PE gather)
                f_stack = ExitStack()
                fsc = f_stack.enter_context(tc.tile_pool(name="fsc", bufs=1))
                fNP = f_stack.enter_context(tc.tile_pool(name="fNP", bufs=1))
                # N' = q*ln(sm) = -N ; P' = e2*(ln(sm)+x) = -P (host negates)
                Nts, Pts = [], []
                for c0 in range(0, C if do_focal else 0, 4):
                    rnd = list(range(c0, min(c0 + 4, C)))
                    sms, qs, e2s = {}, {}, {}
                    for c in rnd:
                        xc = xcs[c]
                        sm = fsc.tile([P, COLS], f16, name=f"sm{c % 4}")
                        nc.scalar.activation(sm[:], xc[:], AF.Sigmoid,
                                             scale=nsc[:])
                        qv = fsc.tile([P, COLS], f16, name=f"qv{c % 4}")
                        nc.scalar.activation(qv[:], sm[:], AF.Square,
                                             scale=nsc[:], bias=ones[:])
                        ev = fsc.tile([P, COLS], f16, name=f"ev{c % 4}")
                        nc.scalar.activation(ev[:], sm[:], AF.Square)
                        sms[c] = sm; qs[c] = qv; e2s[c] = ev
                    for c in rnd:
                        xc = xcs[c]
                        lnsm = fsc.tile([P, COLS], f16, name=f"ln{c % 4}")
                        nc.scalar.activation(lnsm[:], sms[c][:], AF.Ln)
                        Nt = fsc.tile([P, COLS], f16, name="Nt")
                        nc.vector.tensor_tensor(out=Nt[:], in0=qs[c][:],
                                                in1=lnsm[:], op=AL.mult)
                        sc1 = fsc.tile([P, COLS], f16, name="sc1")
                        nc.vector.tensor_scalar(out=sc1[:], in0=Nt[:],
                                                scalar1=1.0, scalar2=None,
                                                op0=AL.mult, op1=AL.add,
                                                accum_out=tac[:])
                        nc.vector.tensor_tensor(out=nsA[:], in0=nsA[:],
                                                in1=tac[:], op=AL.add)
                        bv = fsc.tile([P, COLS], f16, name="bv")
                        Pt = fsc.tile([P, COLS], f16, name="Pt")
                        if c < gp_focal:
                            nc.gpsimd.tensor_tensor(out=bv[:], in0=lnsm[:],
                                                    in1=xc[:], op=AL.add)
                            nc.gpsimd.tensor_tensor(out=Pt[:], in0=e2s[c][:],
                                                    in1=bv[:], op=AL.mult)
                        else:
                            nc.vector.tensor_tensor(out=bv[:], in0=lnsm[:],
                                                    in1=xc[:], op=AL.add)
                            nc.vector.tensor_tensor(out=Pt[:], in0=e2s[c][:],
                                                    in1=bv[:], op=AL.mult)
                        Rt = fNP.tile([P, COLS], f16, name=f"Rt{c}")
                        nc.vector.scalar_tensor_tensor(
                            out=Rt[:], in0=Pt[:], scalar=1.0 / 3.0,
                            in1=Nt[:], op0=AL.mult, op1=AL.subtract)
                        Nts.append(Rt)

                for half in range(2 if do_gather else 0):
                    base = half * WH
                    oh = ohp.tile([P, G * WH], f16, name="oh")
                    oh_ap = oh[:]
                    for gl in range(G):
                        eng_oh = nc.gpsimd if gl >= G - gp_oh else nc.vector
                        dst = dataclasses.replace(
                            oh_ap, offset=oh_ap.offset + gl,
                            ap=[oh_ap.ap[0], [G, WH]])
                        eng_oh.tensor_scalar(
                            out=dst,
                            in0=bpm[:, base:base + WH],
                            scalar1=float(gl + 1), scalar2=None,
                            op0=AL.is_equal)
                    nb = WH // 32          # 32-col groups per half
                    for qb in range(nb):
                        gp = gpp.tile([P, 160], f32, name="gp")
                        for bb in range(2):      # two 16-col batches
                            pt = ptp.tile([P, 512], f16, name="pt")
                            for t4 in range(4):
                                c0 = (qb * 32 + bb * 16 + t4 * 4) * G
                                nc.tensor.transpose(
                                    pt[:, 128 * t4:128 * t4 + 128],
                                    oh[:, c0:c0 + 128], ident[:])
                            ohT = otp.tile([P, 512], f16, name="ohT")
                            if (qb * 2 + bb) % 2 == 0:
                                nc.vector.tensor_copy(ohT[:], pt[:])
                            else:
                                nc.scalar.copy(ohT[:], pt[:])
                            for t4 in range(4):
                                nc.tensor.matmul(
                                    out=gp[:, 80 * bb + 20 * t4:
                                           80 * bb + 20 * t4 + 20],
                                    lhsT=ohT[:, 128 * t4:128 * t4 + 128],
                                    rhs=tt20[:], start=True, stop=True)
                        # scatter gp [P, (8 chunk,4 c,5 v)] -> gath planes
                        gsrcp = gp[:].rearrange("p (k c v) -> p v k c",
                                                k=8, c=4, v=5)
                        gdst = gath[:]
                        gdst_ap = dataclasses.replace(
                            gdst, offset=gdst.offset + base + qb * 32,
                            ap=[gdst.ap[0], [W, 5], [4, 8], [1, 4]])
                        nc.scalar.copy(gdst_ap, gsrcp)

                # ---------- focal part 2: corr' = sum m*R' ----------
                for c in range(C if do_focal else 0):
                    mv = fsc.tile([P, COLS], f16, name="mv")
                    nc.vector.tensor_scalar(out=mv[:], in0=gpl(4),
                                            scalar1=float(c + 1),
                                            scalar2=None, op0=AL.is_equal)
                    t1 = fsc.tile([P, COLS], f16, name="t1")
                    nc.vector.tensor_tensor(out=t1[:], in0=mv[:],
                                            in1=Nts[c][:], op=AL.mult)
                    ta = tac if c % 2 == 0 else tac2
                    ca = cpA if c % 2 == 0 else cnA
                    nc.vector.tensor_scalar(out=t1[:], in0=t1[:], scalar1=1.0,
                                            scalar2=None, op0=AL.mult,
                                            op1=AL.add, accum_out=ta[:])
                    nc.vector.tensor_tensor(out=ca[:], in0=ca[:],
                                            in1=ta[:], op=AL.add)

                f_stack.close()

                # ---------- reg smooth-L1 ----------
                rsc = gctx.enter_context(tc.tile_pool(name="rsc", bufs=2))
                fsc = rsc
                posc = pos[:, 0:COLS]
                for k in range(4 if do_reg else 0):
                    rt = fsc.tile([P, COLS], f16, name="rt")
                    if k == 0:
                        r1 = fsc.tile([P, COLS], f16, name="r1")
                        nc.vector.tensor_tensor(out=r1[:], in0=gpl(0),
                                                in1=xa2[:], op=AL.subtract)
                        nc.vector.tensor_tensor(out=rt[:], in0=r1[:],
                                                in1=iwa2[:], op=AL.mult)
                    elif k == 1:
                        r1 = fsc.tile([P, COLS], f16, name="r1")
                        nc.vector.tensor_tensor(out=r1[:], in0=gpl(1),
                                                in1=ya2[:], op=AL.subtract)
                        nc.vector.tensor_tensor(out=rt[:], in0=r1[:],
                                                in1=iha2[:], op=AL.mult)
                    elif k == 2:
                        nc.vector.tensor_tensor(out=rt[:], in0=gpl(2),
                                                in1=La[:], op=AL.subtract)
                    else:
                        nc.vector.tensor_tensor(out=rt[:], in0=gpl(3),
                                                in1=Ha[:], op=AL.subtract)
                    eng = nc.gpsimd if k < gp_reg else nc.vector
                    e = fsc.tile([P, COLS], f16, name="e")
                    eng.tensor_tensor(out=e[:], in0=rps[k][:], in1=rt[:],
                                      op=AL.subtract)
                    q = fsc.tile([P, COLS], f16, name="q")
                    nc.scalar.activation(q[:], e[:], AF.Abs)
                    qm = fsc.tile([P, COLS], f16, name="qm")
                    eng.tensor_tensor(out=qm[:], in0=q[:], in1=posc,
                                      op=AL.mult)
                    cm = fsc.tile([P, COLS], f16, name="cm")
                    eng.tensor_scalar(out=cm[:], in0=qm[:], scalar1=BETA,
                                      scalar2=None, op0=AL.min)
                    t3 = fsc.tile([P, COLS], f16, name="t3")
                    eng.tensor_scalar(out=t3[:], in0=qm[:], scalar1=2.0,
                                      scalar2=None, op0=AL.mult)
                    eng.tensor_tensor(out=t3[:], in0=t3[:], in1=cm[:],
                                      op=AL.subtract)
                    t4_ = fsc.tile([P, COLS], f16, name="t4_")
                    nc.vector.tensor_tensor(out=t4_[:], in0=cm[:], in1=t3[:],
                                            op=AL.mult)
                    nc.vector.tensor_scalar(out=t4_[:], in0=t4_[:],
                                            scalar1=1.0, scalar2=None,
                                            op0=AL.mult, op1=AL.add,
                                            accum_out=tac[:])
                    nc.vector.tensor_tensor(out=sl1A[:], in0=sl1A[:],
                                            in1=tac[:], op=AL.add)

            # ---------- final cross-partition reduce ----------
            acc6 = persist.tile([P, 6], f32, name="acc6")
            nc.vector.memset(acc6[:], 0.0)
            nc.scalar.copy(acc6[:, 0:1], nposA[:])
            nc.scalar.copy(acc6[:, 1:2], sl1A[:])
            nc.scalar.copy(acc6[:, 2:3], nsA[:])
            nc.scalar.copy(acc6[:, 3:4], cpA[:])
            nc.scalar.copy(acc6[:, 4:5], cnA[:])
            with tc.tile_pool(name="psf", bufs=1, space="PSUM") as pf:
                fps = pf.tile([1, 6], f32, name="fps")
                nc.tensor.matmul(out=fps[:], lhsT=ones[:], rhs=acc6[:],
                                 start=True, stop=True)
                osb = persist.tile([1, 6], f32, name="osb")
                nc.scalar.copy(osb[:], fps[:])
                nc.sync.dma_start(out[:], osb[:])

    return nc


# ---------------- host side ----------------

def pack_inputs(cls_preds, reg_preds, anchors, gt_boxes, gt_labels):
    """Full inputs -> list of 8 per-core input maps (planar fp16 layouts)."""
    B, A, _ = cls_preds.shape
    f = np.float32
    a = anchors.astype(f)
    wa = a[:, 2] - a[:, 0]
    ha = a[:, 3] - a[:, 1]
    anch16 = np.stack([
        a[:, 0], a[:, 1], a[:, 2], a[:, 3],
        a[:, 0] + a[:, 2], a[:, 1] + a[:, 3],
        0.5 / (wa + EPS), 0.5 / (ha + EPS),
        np.log(wa + EPS), np.log(ha + EPS),
    ]).astype(np.float16)
    area = ((wa * ha)[None, :]).astype(f)
    maps = []
    for b in range(B):
        clsp = np.ascontiguousarray(cls_preds[b].astype(np.float16).T)
        regp = np.ascontiguousarray(reg_preds[b].astype(np.float16).T)
        gb = gt_boxes[b].astype(f)
        areaB = (gb[:, 2] - gb[:, 0]) * (gb[:, 3] - gb[:, 1])
        gtbs = np.concatenate([gb[:, 0], gb[:, 1], gb[:, 2], gb[:, 3],
                               areaB]).astype(f)[None, :]
        # PE gather table [128, 20]: row 4g+c -> cols[5c:5c+5] = vals[g]
        vals = np.stack([
            gb[:, 0] + gb[:, 2], gb[:, 1] + gb[:, 3],
            np.log(gb[:, 2] - gb[:, 0]), np.log(gb[:, 3] - gb[:, 1]),
            gt_labels[b].astype(f) + 1.0,
        ], axis=1)                                   # [32, 5]
        tbl = np.zeros((P, 20), np.float16)
        for c in range(4):
            tbl[32 * c:32 * c + 32, 5 * c:5 * c + 5] = vals.astype(np.float16)
        maps.append({"anch": anch16, "area": area, "clsp": clsp,
                     "regp": regp, "gtbs": gtbs, "tbl": tbl})
    return maps


def finish(partials):
    """partials: list of [1,6] arrays per core -> (cls_loss, reg_loss)."""
    f = np.float32
    npos = f(0); sl1 = f(0); nsum = f(0); cp = f(0); cn = f(0)
    for p in partials:
        p = p.reshape(6)
        npos += f(p[0]); sl1 += f(p[1]); nsum += f(p[2])
        cp += f(p[3]); cn += f(p[4])
    # kernel accumulates negated sums: nsum' = -nsum, corr' = cp+cn = -corr
    nsum = -nsum
    corr = -(cp + cn)
    denom = max(float(npos), 1.0)
    if npos > 0:
        cls_loss = f(0.75) * (nsum + corr) / f(denom)
        reg_loss = sl1 / f(2 * BETA) / f(denom)
    else:
        cls_loss = f(0.0); reg_loss = f(0.0)
    return np.float32(cls_loss), np.float32(reg_loss)


# ---------------- self-contained kernel entry ----------------

_CACHE = {}


def _get_fn(n_cores=8):
    """Build + jit the 8-core SPMD executable once."""
    if "fn" in _CACHE:
        return _CACHE["fn"]
    import jax
    from jax.sharding import Mesh, PartitionSpec, NamedSharding
    from jax.experimental.shard_map import shard_map
    from concourse.bass2jax import (_bass_exec_p, install_neuronx_cc_hook,
                                    partition_id_tensor)
    patch_tile_drain(1)
    nc = build(160000)
    split_sync_waits(nc)
    install_neuronx_cc_hook()
    in_names, out_names, out_avals, zero_shapes = [], [], [], []
    partition_name = (nc.partition_id_tensor.name
                      if nc.partition_id_tensor else None)
    for alloc in nc.m.functions[0].allocations:
        if not isinstance(alloc, mybir.MemoryLocationSet):
            continue
        name = alloc.memorylocations[0].name
        if alloc.kind == "ExternalInput":
            if name != partition_name:
                in_names.append(name)
        elif alloc.kind == "ExternalOutput":
            out_names.append(name)
            shape = tuple(alloc.tensor_shape)
            dtype = mybir.dt.np(alloc.dtype)
            out_avals.append(jax.core.ShapedArray(shape, dtype))
            zero_shapes.append((shape, dtype))
    n_params = len(in_names)
    n_outs = len(out_avals)
    all_in_names = in_names + out_names + ([partition_name]
                                           if partition_name else [])
    donate = tuple(range(n_params, n_params + n_outs))

    def _body(*args):
        operands = list(args)
        if partition_name is not None:
            operands.append(partition_id_tensor())
        outs = _bass_exec_p.bind(
            *operands, out_avals=tuple(out_avals),
            in_names=tuple(all_in_names), out_names=tuple(out_names),
            lowering_input_output_aliases=(),
            sim_require_finite=True, sim_require_nnan=True, nc=nc)
        return tuple(outs)

    devices = jax.devices()[:n_cores]
    mesh = Mesh(np.asarray(devices), ("core",))
    in_specs = (PartitionSpec("core"),) * (n_params + n_outs)
    out_specs = (PartitionSpec("core"),) * len(out_names)
    fn = jax.jit(shard_map(_body, mesh=mesh, in_specs=in_specs,
                           out_specs=out_specs, check_rep=False),
                 donate_argnums=donate, keep_unused=True)
    sh = NamedSharding(mesh, PartitionSpec("core"))
    _CACHE["fn"] = (fn, in_names, out_names, out_avals, zero_shapes, sh,
                    n_cores)
    return _CACHE["fn"]


def kernel(cls_preds, reg_preds, anchors, gt_boxes, gt_labels):
    """Full-input DetectionLoss on 8 NeuronCores (data-parallel over batch).

    Returns (cls_loss, reg_loss) as float32 scalars, matching reference()."""
    import jax
    cls_preds = np.asarray(cls_preds)
    reg_preds = np.asarray(reg_preds)
    anchors = np.asarray(anchors)
    gt_boxes = np.asarray(gt_boxes)
    gt_labels = np.asarray(gt_labels)
    B, A, _ = cls_preds.shape
    assert (B, A) == (8, 160000), (B, A)
    maps = pack_inputs(cls_preds, reg_preds, anchors, gt_boxes, gt_labels)
    fn, in_names, out_names, out_avals, zero_shapes, sh, n_cores = _get_fn()
    concat_in = [jax.device_put(
        np.concatenate([np.asarray(maps[c][nm]) for c in range(n_cores)],
                       axis=0), sh) for nm in in_names]
    zeros = [jax.device_put(
        np.zeros((n_cores * s[0], *s[1:]), d), sh) for s, d in zero_shapes]
    out_arrs = fn(*concat_in, *zeros)
    res = np.asarray(out_arrs[out_names.index("out")]).reshape(n_cores, 1, 6)
    partials = [res[c] for c in range(n_cores)]
    cls_loss, reg_loss = finish(partials)
    return cls_loss, reg_loss
